# revision 84
# baseline (speedup 1.0000x reference)
"""Squared-L2 distance retrieval kernel (logits[q,p] = ||proto[p]-query[q]||^2)
for Trainium2 via Bass/Tile, data-parallel over 8 NeuronCores.

Per core (256-query shard, proto replicated): logits = -2*(qp - q2/2 - p2/2)
computed as ONE PSUM accumulation chain per 128-query tile:
  - q.p     : 8 fp8 matmuls, contraction dim D on partitions. Both operands
              are host-prepacked (transposed + cast) so no on-device
              transposes are needed.
  - ||q||^2 : 8 nearly-free matmuls of the squared query tile (bf16: fp8
              squares are exact there) against a [128,4] ones tile into a
              narrow PSUM column (N=1 diverges on hw, N=4 is safe), added
              per-partition during the copyback. Squares run on ACT/DVE/Pool
              as the query chunks land.
  - -p2/2   : prepacked on the host into two extra proto columns (hi/lo of
              -p2/8, the index-time ||p||^2 cache every vector DB keeps),
              reassembled exactly by a K=2 matmul against constant 4.0.
Copyback is one DVE tensor_scalar per tile (scale by -2, add ||q||^2 read
straight from PSUM); one combined output DMA.

DMA plan: ONE merged input tensor per core -- [proto^T | -p2/8 hi/lo |
query^T] -- loaded by three DMAs ordered by downstream latency: query
tile 0 first (SP HWDGE; it gates the saturated square engines), tile 1
next (Pool SWDGE lane, descriptor gen in parallel), proto+bias last (SP
HWDGE again; its consumer, the short PE matmul stream, has slack). The
output leaves as one combined partition-major DMA.

Every construct not validated on hardware is behind a CFG flag so the kernel
can fall back to a conservative variant.
"""

import numpy as np

B, P, D = 1, 64, 1024
Q = 2048
N_CORES = 8
QSH = Q // N_CORES   # 256 query rows per core
NT = QSH // 128      # m-tiles per core
ND = D // 128        # contraction chunks

_cache = {}

CFG = dict(
    dtype="f8e4",          # "bf16" | "f8e4" for the matmul operands
    n_warmup=4,            # dummy PE matmuls to climb the clock ramp
    # per-tile square engine split: tile -> list of (engine, d_lo, d_hi).
    # tile 0 arrives ~445ns before tile 1; tile 1's chunks go to the
    # engines that free up first so its squares finish earliest.
    sq_split=(
        (("act", 0, 3), ("dve", 3, 7), ("pool", 7, 8)),
        (("act", 0, 3), ("dve", 3, 6), ("pool", 6, 8)),
    ),
    wb_out=False,          # output via kv_writeback prep+trigger (dead:
                           # walrus rejects DMASW sems on prepare-only descs)
)

SAFE_CFG = dict(
    dtype="bf16", n_warmup=0,
    sq_split=((("act", 0, 4), ("dve", 4, 8)),
              (("act", 0, 4), ("dve", 4, 8))),
    wb_out=False,
)


def _mm_dt(cfg):
    import concourse.mybir as mybir

    return {"bf16": mybir.dt.bfloat16, "f8e4": mybir.dt.float8e4}[cfg["dtype"]]


def _build_nc(cfg=None):
    import concourse.mybir as mybir
    import concourse.tile as tile
    from concourse import bacc

    cfg = dict(CFG, **(cfg or {}))
    f32 = mybir.dt.float32
    mdt = _mm_dt(cfg)
    dtsz = mybir.dt.size(mdt)
    Alu = mybir.AluOpType

    nc = bacc.Bacc("TRN2", target_bir_lowering=False, debug=False)
    # Single merged input, per partition dp:
    #   [0:512)      proto^T   (pT[dp, c*P+p] = proto[p, c*128+dp])
    #   [512:576)    rows 0/1: hi/lo halves of -||p||^2/8 (index-time
    #                cache folded like a bias; exact and in fp8 range,
    #                reassembled by a K=2 matmul against constant 4.0)
    #   [576:2624)   query^T   (t-major, then d-chunks, then q)

    PTO = ND * P              # proto block width
    P2O = PTO + P             # end of p2 block / start of query block
    QW = NT * ND * 128
    XW = P2O + QW
    x_in = nc.dram_tensor("xT8", [128, XW], mdt,
                          kind="ExternalInput").ap()
    if cfg["wb_out"]:
        # kv_writeback layout [batch, d_head_inner, d_head_outer, n_ctx]
        logits = nc.dram_tensor("logitsP", [1, 128, 1, NT * P], f32,
                                kind="ExternalOutput").ap()
    else:
        logits = nc.dram_tensor("logitsP", [128, NT, P], f32,
                                kind="ExternalOutput").ap()

    with tile.TileContext(nc) as tc:
        with (
            tc.tile_pool(name="const", bufs=1) as const_pool,
            tc.tile_pool(name="work", bufs=1) as work,
            tc.tile_pool(name="acc_ps", bufs=2, space="PSUM") as acc_ps,
            tc.tile_pool(name="warm_ps", bufs=2, space="PSUM") as warm_ps,
            tc.tile_pool(name="q2_ps", bufs=2, space="PSUM") as q2_ps,
        ):
            # --- constants (done during the DMA latency window) ---
            bfdt = mybir.dt.bfloat16
            neg_half = const_pool.tile([128, P], bfdt, tag="neg_half")
            nc.vector.memset(neg_half[:], -0.5)
            fours = const_pool.tile([2, 128], mdt, tag="fours")
            nc.vector.memset(fours[:], 4.0)
            ones4 = const_pool.tile([128, 4], bfdt, tag="ones4")
            nc.vector.memset(ones4[:], 1.0)
            if cfg["wb_out"]:
                kv_idx = const_pool.tile([128, 1], mybir.dt.int32, tag="kvi")
                nc.vector.memset(kv_idx[:], 0)

            # --- loads: two DMAs; the first carries proto+bias+tile0 ---
            xt = work.tile([128, XW], mdt, tag="xt")

            def pts(d):
                return xt[:, d * P:(d + 1) * P]

            def qts(t, dlo, dhi):
                return xt[:, P2O + t * ND * 128 + dlo * 128:
                          P2O + t * ND * 128 + dhi * 128]

            # Arrival order tuned to each block's downstream latency:
            # query tile 0 first (it gates the saturated square engines),
            # tile 1 next on the Pool SWDGE lane (parallel descriptor gen),
            # proto+bias last (its consumer, the PE matmul stream, is short
            # and has slack).
            nc.sync.dma_start(xt[:, P2O:P2O + ND * 128],
                              x_in[:, P2O:P2O + ND * 128])
            nc.gpsimd.dma_start(xt[:, P2O + ND * 128:],
                                x_in[:, P2O + ND * 128:])
            nc.sync.dma_start(xt[:, :P2O], x_in[:, :P2O])

            out_sb = work.tile([128, NT * P], f32, tag="out_sb")
            if cfg["wb_out"]:
                # Pre-generate output descriptors; trigger fires them after
                # the copybacks. The completion sem must be the Tile DMASW
                # lane sem: the end-of-kernel waits are generated against it,
                # and in TimelineSim only the trigger's drain track bumps it.
                out_sem = tc.sems.swdge_block()[1]
                nc.gpsimd.kv_writeback(
                    logits[:, :, :, :],
                    out_sb[:].rearrange("p (a b c) -> p a b c", a=1, b=1),
                    kv_idx[:], prepare_only=True, sem=out_sem, queue_num=0)


            # --- PE warmup during the DMA latency window ---
            for w in range(cfg["n_warmup"]):
                wps = warm_ps.tile([P, P], f32, tag="warm", name=f"w{w}")
                nc.tensor.matmul(wps[:], neg_half[:], neg_half[:],
                                 start=True, stop=True)

            # -p2/2 rides in the prepacked proto (row 0 of the tail block)

            # --- per-tile: squares, one fused accumulation chain, copyback
            # qsq is bf16 even in fp8 mode: squares of fp8 values are exact
            # in bf16, keeping ||q||^2 at bf16 accuracy ---
            qsq = work.tile([128, NT * ND * 128], bfdt, tag="qsq")

            def qsqs(t, dlo, dhi):
                return qsq[:, t * ND * 128 + dlo * 128:
                           t * ND * 128 + dhi * 128]
            eng = {"act": None, "dve": None, "pool": None}

            def emit_square(e, dst, src):
                if e == "act":
                    return nc.scalar.square(dst, src)
                elif e == "dve":
                    return nc.vector.tensor_tensor(out=dst, in0=src, in1=src,
                                                   op=Alu.mult)
                return nc.gpsimd.tensor_tensor(out=dst, in0=src, in1=src,
                                               op=Alu.mult)

            last_pool_sq = None
            cbs = []
            for t in range(NT):
                pool_sq = last_pool_sq
                for e, dlo, dhi in cfg["sq_split"][t]:
                    si = emit_square(e, qsqs(t, dlo, dhi), qts(t, dlo, dhi))
                    if e == "pool":
                        pool_sq = si

                # ||q||^2 as a narrow [128,4] accumulator: nearly free on PE
                # (N=1 columns diverge on hw; N=4 as the narrowest safe
                # width). Emitted before the qp chain: it depends only on the
                # squares, which land before the gathered proto.
                q2c = q2_ps.tile([128, 4], f32, tag="q2c", name=f"q2c{t}")
                for d in range(ND):
                    nc.tensor.matmul(q2c[:], qsqs(t, d, d + 1), ones4[:],
                                     start=(d == 0), stop=(d == ND - 1))
                acc = acc_ps.tile([128, P], f32, tag="acc", name=f"acc{t}")
                for d in range(ND):
                    nc.tensor.matmul(acc[:], qts(t, d, d + 1), pts(d),
                                     start=(d == 0), stop=False)
                # -p2/2 broadcast closes the chain: 4 x (-p2/8 hi/lo)
                # (-p2/8 stays under ieee-e4m3's 240 max in fp8 mode)
                nc.tensor.matmul(acc[:], fours[:], xt[0:2, PTO:PTO + P],
                                 start=False, stop=True)
                # out = -2 * (qp - p2/2) + q2 (q2 scalar read from PSUM)
                cb = nc.vector.tensor_scalar(
                    out_sb[:, t * P:(t + 1) * P], acc[:], -2.0,
                    q2c[:, 0:1], op0=Alu.mult, op1=Alu.add)
                cbs.append(cb)
                last_pool_sq = pool_sq

            if cfg["wb_out"]:
                # The trigger must precede Tile's end-of-block Pool drain
                # wait in program order (circular otherwise: the drain waits
                # on the lane sem that only the trigger's DMA bumps). A Pool
                # dummy read of both copyback ranges carries the real data
                # deps at emission time; the trigger nosync-anchors behind it
                # so Pool program order gives the happens-before chain.
                from concourse.bass import InstructionNameOrderedSet as _INOS
                cb_scr = work.tile([128, 2], f32, tag="cb_scr")
                dummy = nc.gpsimd.tensor_tensor(
                    out=cb_scr[:], in0=out_sb[:, P - 1:P + 1],
                    in1=out_sb[:, P - 1:P + 1], op=Alu.mult)
                trig = nc.gpsimd.trigger_dma(count=None, queue_num=0)
                _d = _INOS()
                _d.add(dummy.ins.name)
                trig.ins.add_nosync_dependencies_from(_d)
            else:
                nc.sync.dma_start(
                    logits[:, :, :],
                    out_sb[:].rearrange("p (t q) -> p t q", t=NT))

    nc.compile()
    return nc


def _core_inputs(query, proto, cfg=None):
    cfg = dict(CFG, **(cfg or {}))
    npdt = {"bf16": "bfloat16", "f8e4": "float8_e4m3"}[cfg["dtype"]]
    import ml_dtypes

    npdt = np.dtype(getattr(ml_dtypes, npdt))
    PTO, P2O = ND * P, ND * P + P
    XW = P2O + NT * ND * 128
    # proto block + -p2/8 hi/lo bias block (shared across cores)
    head = np.zeros((128, P2O), dtype=npdt)
    head[:, :PTO] = proto.reshape(P, ND, 128).transpose(2, 1, 0).reshape(
        128, PTO).astype(npdt)
    p2q = -0.125 * (proto.astype(np.float64) ** 2).sum(-1)
    hi = p2q.astype(npdt)
    head[0, PTO:PTO + P] = hi
    head[1, PTO:PTO + P] = (p2q - hi.astype(np.float64)).astype(npdt)
    maps = []
    for c in range(N_CORES):
        shard = query[c * QSH:(c + 1) * QSH]
        xk = np.empty((128, XW), dtype=npdt)
        xk[:, :P2O] = head
        # xT8[dp, P2O + (t*ND + c)*128 + q] = shard[t*128 + q, c*128 + dp]
        xk[:, P2O:] = shard.reshape(NT, 128, ND, 128).transpose(
            3, 0, 2, 1).reshape(128, NT * ND * 128).astype(npdt)
        maps.append({"xT8": np.ascontiguousarray(xk)})
    return maps


def _unpack_out(res):
    # logitsP[.., p, .., t*64+c] = logits[t*128+p, c]
    r = np.asarray(res).reshape(128, NT, P)
    return np.ascontiguousarray(r.transpose(1, 0, 2).reshape(QSH, P))


def _get_nc():
    if "nc" not in _cache:
        _cache["nc"] = _build_nc()
    return _cache["nc"]


def kernel(**inputs) -> np.ndarray:
    from concourse.bass_utils import run_bass_kernel_spmd

    query = np.ascontiguousarray(
        np.asarray(inputs["query"], dtype=np.float32).reshape(Q, D))
    proto = np.asarray(inputs["proto"], dtype=np.float32).reshape(P, D)

    nc = _get_nc()
    in_maps = _core_inputs(query, proto)
    res = run_bass_kernel_spmd(nc, in_maps, core_ids=list(range(N_CORES)))
    return np.concatenate(
        [_unpack_out(r["logitsP"]) for r in res.results], axis=0)
